# revision 1
# baseline (speedup 1.0000x reference)
"""CrissCross (axial) attention kernel for 8 TRN2 NeuronCores.

Shapes (hardcoded): x [16, 512, 64, 64], Wq/Wk [64, 512], Wv [512, 512].
Sharding: data-parallel over batch, 2 batches per core.

Per-core, per-batch pipeline:
  1. qk = [Wq;Wk] @ x  (fp32r matmuls, bias fused into psum->sbuf copy, bf16)
  2. energies per column w / per row h, bf16, two orientations at once
     (orientation-2 [g,h]/[u,w] -> exp -> P tiles, the aggregation lhsT;
      orientation-1 [h,g]/[w,u] -> exp -> free-axis reduce -> softmax sums).
     No max subtraction (|e| << 80 so fp32 exp is safe). The reference's -inf
     diag mask becomes zeroing the diag of the column part post-exp.
  3. R = 1/(Sc+Sr) assembled via tiny DMAs + PE transposes in both packings.
  4. vT = (gamma*Wv @ x)^T in both spatial orders, via projection matmuls
     whose lhsT M-dim is a (strided) spatial slice of x. fp32r, bf16 out.
  5. aggregation out[h,c] = P^T @ vT per column/row, 2-way packed on the PE
     via partition halves; 1/S fused into the psum->sbuf copy as a
     per-partition scale.
  6. PE-transpose the aggregated parts to channel-major, accumulating both
     parts directly into one t buffer (column part written through a strided
     AP that undoes the (w,h) ordering; row part added on top).
  7. out = t + x via gpsimd, DMA out.
"""

import json
import sys

import ml_dtypes
import numpy as np

sys.path.insert(0, "/root/.axon_site")

from contextlib import ExitStack

import concourse.bass as bass
import concourse.bass2jax as b2j
import concourse.mybir as mybir
import concourse.tile as tile
from concourse.bass_utils import run_bass_kernel_spmd

F32 = mybir.dt.float32
F32R = mybir.dt.float32r
BF16 = mybir.dt.bfloat16
AF = mybir.ActivationFunctionType
NE = mybir.AluOpType.not_equal

B, C, H, W = 16, 512, 64, 64
S = H * W            # 4096
NB = 2               # batches per core
NCORES = 8

_PATCHED = False


def _install_multiwait_split():
    """This container's walrus rejects instructions carrying >1 sem waits.
    Split extras into standalone EventSemaphore waits on the same engine,
    inserted immediately before (preserves per-engine program order)."""
    global _PATCHED
    if _PATCHED:
        return
    _PATCHED = True
    orig = b2j._decompress_ant_bir

    def _split(s):
        d = json.loads(orig(s))
        for fn in d.get("functions", []):
            for blk in fn.get("blocks", []):
                out = []
                for ins in blk.get("instructions", []):
                    si = ins.get("sync_info")
                    ow = (si or {}).get("on_wait") or []
                    if len(ow) > 1:
                        for i, w in enumerate(ow[:-1]):
                            out.append({
                                "debug": ins.get("debug", 0),
                                "engine": ins["engine"],
                                "ins": [], "outs": [],
                                "name": f'{ins["name"]}-xw{i}',
                                "opcode": "EventSemaphore",
                                "sync_info": {"on_update": [], "on_wait": [w]},
                            })
                        si["on_wait"] = [ow[-1]]
                    out.append(ins)
                blk["instructions"] = out
        return json.dumps(d).encode()

    b2j._decompress_ant_bir = _split


def build_nc():
    nc = bass.Bass("TRN2", target_bir_lowering=False, debug=False)

    x_d = nc.dram_tensor("x", [NB, C, S], F32R, kind="ExternalInput").ap()
    wqk_d = nc.dram_tensor("wqkT", [C, 128], F32R, kind="ExternalInput").ap()
    bqk_d = nc.dram_tensor("bqk", [128, 1], F32, kind="ExternalInput").ap()
    wv_d = nc.dram_tensor("wvT", [C, C], F32R, kind="ExternalInput").ap()
    gbv_d = nc.dram_tensor("gbv", [C, 1], F32, kind="ExternalInput").ap()
    idf_d = nc.dram_tensor("identf", [128, 128], F32, kind="ExternalInput").ap()
    idb_d = nc.dram_tensor("identb", [128, 128], BF16, kind="ExternalInput").ap()
    out_d = nc.dram_tensor("out", [NB, C, S], F32, kind="ExternalOutput").ap()

    x_v = x_d.rearrange("b (kc p) s -> b p kc s", p=128)
    out_v = out_d.rearrange("b (kc p) s -> b p kc s", p=128)
    wqk_v = wqk_d.rearrange("(kc p) m -> p kc m", p=128)
    wv_v = wv_d.rearrange("(kc p) m -> p kc m", p=128)
    gbv_v = gbv_d.rearrange("(kc p) one -> p kc one", p=128)

    with tile.TileContext(nc) as tc, ExitStack() as ctx:
        consts = ctx.enter_context(tc.tile_pool(name="consts", bufs=1))
        wqk_sb = consts.tile([128, 4, 128], F32R)
        nc.sync.dma_start(out=wqk_sb, in_=wqk_v)
        wv_sb = consts.tile([128, 4, 512], F32R)
        nc.sync.dma_start(out=wv_sb, in_=wv_v)
        bqk_sb = consts.tile([128, 1], F32)
        nc.sync.dma_start(out=bqk_sb, in_=bqk_d)
        gbv_sb = consts.tile([128, 4, 1], F32)
        nc.sync.dma_start(out=gbv_sb, in_=gbv_v)
        idf_sb = consts.tile([128, 128], F32)
        nc.sync.dma_start(out=idf_sb, in_=idf_d)
        idb_sb = consts.tile([128, 128], BF16)
        nc.sync.dma_start(out=idb_sb, in_=idb_d)

        # psum pools (8 banks total)
        pp_proj = ctx.enter_context(tc.tile_pool(name="pp_proj", bufs=2, space="PSUM"))
        pp_en = ctx.enter_context(tc.tile_pool(name="pp_en", bufs=1, space="PSUM"))
        pp_agg = ctx.enter_context(tc.tile_pool(name="pp_agg", bufs=2, space="PSUM"))
        pp_tr = ctx.enter_context(tc.tile_pool(name="pp_tr", bufs=1, space="PSUM"))
        pp_sm = ctx.enter_context(tc.tile_pool(name="pp_sm", bufs=1, space="PSUM"))

        px = ctx.enter_context(tc.tile_pool(name="px", bufs=1))
        pqk = ctx.enter_context(tc.tile_pool(name="pqk", bufs=1))
        pP = ctx.enter_context(tc.tile_pool(name="pP", bufs=1))
        pscr = ctx.enter_context(tc.tile_pool(name="pscr", bufs=1))
        pst = ctx.enter_context(tc.tile_pool(name="pst", bufs=2))
        pvt = ctx.enter_context(tc.tile_pool(name="pvt", bufs=1))
        put = ctx.enter_context(tc.tile_pool(name="put", bufs=1))
        pt = ctx.enter_context(tc.tile_pool(name="pt", bufs=1))
        po = ctx.enter_context(tc.tile_pool(name="po", bufs=1))

        for b in range(NB):
            # ---- load x ----------------------------------------------------
            x_sb = px.tile([128, 4, S], F32R, tag="x")
            for kc in range(4):
                nc.sync.dma_start(out=x_sb[:, kc, :], in_=x_v[b, :, kc, :])

            # ---- qk projection --------------------------------------------
            qkA = po.tile([128, S], BF16, tag="o")
            qkB = pt.tile([128, S], BF16, tag="t")
            for n in range(8):
                ps = pp_proj.tile([128, 512], F32, tag="proj")
                for kc in range(4):
                    nc.tensor.matmul(
                        ps,
                        lhsT=wqk_sb[:, kc, :],
                        rhs=x_sb[:, kc, n * 512:(n + 1) * 512],
                        start=(kc == 0), stop=(kc == 3),
                    )
                nc.scalar.activation(
                    out=qkA[:, n * 512:(n + 1) * 512], in_=ps,
                    func=AF.Identity, bias=bqk_sb, scale=1.0,
                )
            # B = partition-swapped copy of A (k on top, q on bottom)
            nc.sync.dma_start(out=qkB[0:64, :], in_=qkA[64:128, :])
            nc.sync.dma_start(out=qkB[64:128, :], in_=qkA[0:64, :])

            Acol = qkA.rearrange("p (h w) -> p w h", w=W)
            Bcol = qkB.rearrange("p (h w) -> p w h", w=W)
            Arow = qkA.rearrange("p (h w) -> p h w", h=H)
            Brow = qkB.rearrange("p (h w) -> p h w", h=H)

            # ---- energies + softmax sums ----------------------------------
            PcolT = pP.tile([128, 2048], BF16, tag="PcolT")
            ProwT = pP.tile([128, 2048], BF16, tag="ProwT")
            Sc_p = pst.tile([128, 32], F32, tag="Scp")
            Sr_p = pst.tile([128, 32], F32, tag="Srp")

            for part in ("col", "row"):
                P_sb = PcolT if part == "col" else ProwT
                S_sb = Sc_p if part == "col" else Sr_p
                Ksrc = Bcol if part == "col" else Brow   # k in top half
                Qsrc = Acol if part == "col" else Arow   # q in top half
                for bi in range(4):
                    o2 = pp_en.tile([128, 512], F32, tag="o2")
                    o1 = pp_en.tile([128, 512], F32, tag="o1")
                    for sl in range(8):
                        m = bi * 8 + sl
                        w0, w1 = 2 * m, 2 * m + 1
                        fs = slice(sl * 64, (sl + 1) * 64)
                        # orientation-2: out [g, h] (resp. [u, w])
                        nc.tensor.matmul(o2[0:64, fs], lhsT=Ksrc[0:64, w0, :],
                                         rhs=Qsrc[0:64, w0, :], start=True, stop=True)
                        nc.tensor.matmul(o2[64:128, fs], lhsT=Qsrc[64:128, w1, :],
                                         rhs=Ksrc[64:128, w1, :], start=True, stop=True)
                        # orientation-1: out [h, g] (resp. [w, u])
                        nc.tensor.matmul(o1[0:64, fs], lhsT=Qsrc[0:64, w0, :],
                                         rhs=Ksrc[0:64, w0, :], start=True, stop=True)
                        nc.tensor.matmul(o1[64:128, fs], lhsT=Ksrc[64:128, w1, :],
                                         rhs=Qsrc[64:128, w1, :], start=True, stop=True)
                    bs = slice(bi * 512, (bi + 1) * 512)
                    nc.scalar.activation(out=P_sb[:, bs], in_=o2, func=AF.Exp)
                    scr = pscr.tile([128, 512], F32, tag="scr")
                    nc.scalar.activation(out=scr, in_=o1, func=AF.Exp)
                    if part == "col":
                        scr3 = scr.rearrange("p (m g) -> p m g", g=64)
                        nc.gpsimd.affine_select(
                            out=scr3[0:64], in_=scr3[0:64],
                            pattern=[[0, 8], [-1, 64]], compare_op=NE,
                            fill=0.0, base=0, channel_multiplier=1)
                        nc.gpsimd.affine_select(
                            out=scr3[64:128], in_=scr3[64:128],
                            pattern=[[0, 8], [-1, 64]], compare_op=NE,
                            fill=0.0, base=0, channel_multiplier=1)
                    nc.vector.reduce_sum(
                        out=S_sb[:, bi * 8:(bi + 1) * 8],
                        in_=scr.rearrange("p (m g) -> p m g", g=64),
                        axis=mybir.AxisListType.X)
                if part == "col":
                    P3 = P_sb.rearrange("p (m h) -> p m h", h=64)
                    nc.gpsimd.affine_select(
                        out=P3[0:64], in_=P3[0:64],
                        pattern=[[0, 32], [-1, 64]], compare_op=NE,
                        fill=0.0, base=0, channel_multiplier=1)
                    nc.gpsimd.affine_select(
                        out=P3[64:128], in_=P3[64:128],
                        pattern=[[0, 32], [-1, 64]], compare_op=NE,
                        fill=0.0, base=0, channel_multiplier=1)


            # ---- per spatial order: project vT, aggregate, transpose ------
            # t accumulates the channel-major attention output (bf16).
            t_sb = pt.tile([128, 4, S], BF16, tag="t")
            xw = x_sb.rearrange("p kc (h w) -> p kc w h", w=W)
            for part in ("col", "row"):
                vt = pvt.tile([128, 32, 512], BF16, tag="vt")
                for j in range(32):
                    ps = pp_proj.tile([128, 512], F32, tag="proj")
                    if part == "col":
                        # walrus rejects dst-partition-64 matmuls with full-K
                        # rows; compute both halves at base 0 and DMA-shift
                        # the odd half into vt[64:128].
                        ps2 = pp_en.tile([64, 512], F32, tag="o2")
                        for kc in range(4):
                            nc.tensor.matmul(
                                ps[0:64, :], lhsT=xw[:, kc, 2 * j, :],
                                rhs=wv_sb[:, kc, :],
                                start=(kc == 0), stop=(kc == 3))
                            nc.tensor.matmul(
                                ps2, lhsT=xw[:, kc, 2 * j + 1, :],
                                rhs=wv_sb[:, kc, :],
                                start=(kc == 0), stop=(kc == 3))
                        stg = pst.tile([64, 512], BF16, tag="stg")
                        nc.vector.tensor_copy(stg, ps2)
                        nc.gpsimd.dma_start(out=vt[64:128, j, :], in_=stg)
                        sl_copy = vt[0:64, j, :]
                        if j % 2 == 0:
                            nc.vector.tensor_copy(sl_copy, ps[0:64, :])
                        else:
                            nc.scalar.activation(out=sl_copy, in_=ps[0:64, :],
                                                 func=AF.Identity)
                        continue
                    if True:
                        for kc in range(4):
                            nc.tensor.matmul(
                                ps, lhsT=x_sb[:, kc, j * 128:(j + 1) * 128],
                                rhs=wv_sb[:, kc, :],
                                start=(kc == 0), stop=(kc == 3))
                    if j % 2 == 0:
                        nc.scalar.activation(out=vt[:, j, :], in_=ps, func=AF.Identity)
                    else:
                        nc.vector.tensor_copy(vt[:, j, :], ps)

                if part == "col":
                    # ---- stats: R = 1/(Sc + Sr) in both pack layouts ---------------
                    Sc_pl = pst.tile([64, 64], F32, tag="Scpl")   # [h, w]
                    Sr_pl = pst.tile([64, 64], F32, tag="Srpl")   # [w, h]
                    Sc2 = Sc_pl.rearrange("p (m t) -> p m t", t=2)
                    Sr2 = Sr_pl.rearrange("p (m t) -> p m t", t=2)
                    nc.sync.dma_start(out=Sc2[:, :, 0], in_=Sc_p[0:64, :])
                    nc.sync.dma_start(out=Sc2[:, :, 1], in_=Sc_p[64:128, :])
                    nc.sync.dma_start(out=Sr2[:, :, 0], in_=Sr_p[0:64, :])
                    nc.sync.dma_start(out=Sr2[:, :, 1], in_=Sr_p[64:128, :])
                    tp = pp_sm.tile([64, 64], F32, tag="stps")
                    nc.tensor.transpose(tp, Sr_pl, idf_sb[0:64, 0:64])   # -> [h, w]
                    R_hw = pst.tile([64, 64], F32, tag="Rhw")
                    nc.vector.tensor_add(R_hw, Sc_pl, tp)
                    nc.vector.reciprocal(out=R_hw, in_=R_hw)
                    tp2 = pp_sm.tile([64, 64], F32, tag="stps")
                    nc.tensor.transpose(tp2, R_hw, idf_sb[0:64, 0:64])   # -> [w, h]
                    R_wh = pst.tile([64, 64], F32, tag="Rwh")
                    nc.vector.tensor_copy(R_wh, tp2)
                    R_cs = pst.tile([128, 32], F32, tag="Rcs")
                    R_rs = pst.tile([128, 32], F32, tag="Rrs")
                    Rhw2 = R_hw.rearrange("p (m t) -> p m t", t=2)
                    Rwh2 = R_wh.rearrange("p (m t) -> p m t", t=2)
                    nc.sync.dma_start(out=R_cs[0:64, :], in_=Rhw2[:, :, 0])
                    nc.sync.dma_start(out=R_cs[64:128, :], in_=Rhw2[:, :, 1])
                    nc.sync.dma_start(out=R_rs[0:64, :], in_=Rwh2[:, :, 0])
                    nc.sync.dma_start(out=R_rs[64:128, :], in_=Rwh2[:, :, 1])

                P_sb = PcolT if part == "col" else ProwT
                R_sb = R_cs if part == "col" else R_rs
                ut = put.tile([128, 32, 512], BF16, tag="ut")
                for m in range(32):
                    aps = pp_agg.tile([128, 512], F32, tag="agg")
                    nc.tensor.matmul(aps[0:64, :],
                                     lhsT=P_sb[0:64, m * 64:(m + 1) * 64],
                                     rhs=vt[0:64, m, :], start=True, stop=True)
                    nc.tensor.matmul(aps[64:128, :],
                                     lhsT=P_sb[64:128, m * 64:(m + 1) * 64],
                                     rhs=vt[64:128, m, :], start=True, stop=True)
                    if m % 2 == 0:
                        nc.scalar.activation(out=ut[:, m, :], in_=aps,
                                             func=AF.Copy, scale=R_sb[:, m:m + 1])
                    else:
                        nc.vector.tensor_scalar_mul(out=ut[:, m, :], in0=aps,
                                                    scalar1=R_sb[:, m:m + 1])

                # PE-transpose to channel-major, accumulate into t
                for cc in range(4):
                    for g4 in range(8):
                        tps = pp_tr.tile([128, 512], BF16, tag="tr")
                        for mm in range(4):
                            m = g4 * 4 + mm
                            nc.tensor.transpose(
                                tps[:, mm * 128:(mm + 1) * 128],
                                ut[:, m, cc * 128:(cc + 1) * 128], idb_sb)
                        if part == "col":
                            # write with reorder: (w,h)-ordered data -> (h,w)
                            # src block mm holds s2 in [m*128,(m+1)*128):
                            # col j2 = h + 64*tpar, w = 2m + tpar
                            src = tps.rearrange("p (mm tpar h) -> p mm tpar h",
                                                mm=4, tpar=2)
                            dst = t_sb[:, cc, :].rearrange(
                                "p (h wq mm tpar) -> p mm tpar h wq",
                                h=64, wq=8, mm=4)[:, :, :, :, g4]
                            nc.scalar.activation(out=dst, in_=src, func=AF.Identity,
                                                 bias=gbv_sb[:, cc, :])
                        else:
                            # row part: contiguous slice, add on top
                            dst = t_sb[:, cc, g4 * 512:(g4 + 1) * 512]
                            nc.vector.tensor_add(dst, tps, dst)

            # ---- final: out = t + x ---------------------------------------
            for cc in range(4):
                for hh in range(2):
                    fs = slice(hh * 2048, (hh + 1) * 2048)
                    o_sb = po.tile([128, 2048], F32, tag="o")
                    nc.gpsimd.tensor_add(o_sb, t_sb[:, cc, fs],
                                         x_sb[:, cc, fs].bitcast(F32))
                    nc.sync.dma_start(out=out_v[b, :, cc, fs], in_=o_sb)

    return nc


_NC = None


def kernel(x, Wq, bq, Wk, bk, Wv, bv, gamma):
    global _NC
    _install_multiwait_split()
    x = np.ascontiguousarray(np.asarray(x, dtype=np.float32))
    Wq = np.asarray(Wq, np.float32); Wk = np.asarray(Wk, np.float32)
    Wv = np.asarray(Wv, np.float32)
    bq = np.asarray(bq, np.float32); bk = np.asarray(bk, np.float32)
    bv = np.asarray(bv, np.float32); gamma = np.asarray(gamma, np.float32)

    wqkT = np.ascontiguousarray(np.concatenate([Wq, Wk], 0).T)       # [512, 128]
    bqk = np.ascontiguousarray(np.concatenate([bq, bk])[:, None])    # [128, 1]
    wvT = np.ascontiguousarray((gamma[0] * Wv).T)                    # [512, 512]
    gbv = np.ascontiguousarray((gamma[0] * bv)[:, None])             # [512, 1]
    identf = np.eye(128, dtype=np.float32)
    identb = np.eye(128, dtype=np.float32).astype(ml_dtypes.bfloat16)

    if _NC is None:
        _NC = build_nc()

    xs = x.reshape(B, C, S)
    in_maps = []
    for i in range(NCORES):
        in_maps.append({
            "x": np.ascontiguousarray(xs[i * NB:(i + 1) * NB]),
            "wqkT": wqkT, "bqk": bqk, "wvT": wvT, "gbv": gbv,
            "identf": identf, "identb": identb,
        })
    res = run_bass_kernel_spmd(_NC, in_maps, list(range(NCORES)))
    out = np.empty((B, C, S), np.float32)
    for i in range(NCORES):
        out[i * NB:(i + 1) * NB] = res.results[i]["out"]
    return out.reshape(B, C, H, W)



# revision 5
# speedup vs baseline: 1.6910x; 1.6910x over previous
"""CrissCross (axial) attention kernel for 8 TRN2 NeuronCores.

Shapes (hardcoded): x [16, 512, 64, 64], Wq/Wk [64, 512], Wv [512, 512].
Sharding: data-parallel over batch, 2 batches per core.

Per-core, per-batch pipeline (all-bf16 operands, fp32 psum):
  1. qk = [Wq;Wk] @ x  (bias fused into psum->sbuf copy).
  2. vT row-major via projection matmuls whose lhsT is a spatial slice of x
     (v bias NOT added here: since the joint softmax weights sum to 1, the
     bias contributes exactly gamma*bv once, added in the final pass).
  3. vT col-major = spatial (h,w)->(w,h) permute of vT via DRAM round-trip
     (plain write + 2 strided readbacks; DMA only, no PE).
  4. energies per column w / per row h, two orientations
     (orientation-2 [g,h]/[u,w] -> exp -> P tiles, the aggregation lhsT;
      orientation-1 [h,g]/[w,u] -> exp -> free-axis reduce -> softmax sums).
     No max subtraction (|e| << 80). Diagonal mask = zero diag post-exp.
  5. R = 1/(Sc+Sr) assembled via tiny DMAs + PE transposes in both packings.
  6. col aggregation out_colT[h,c] = PcolT^T @ vt_cm per w-pair, UNSCALED,
     evacuated bf16, then spatially permuted to row-major order via a second
     DRAM round-trip (permuted write + plain readback).
  7. row aggregation psum = ProwT^T @ vt_rm + I^T @ ot_colp (the permuted col
     part is accumulated into the same psum by an identity matmul), then one
     scaled copy by R (per-partition scalar) -> acc [s, c] bf16.
  8. final: acc^T per 128-block via normal matmul with identity rhs, x folded
     into the same psum by an identity-lhsT matmul, evacuated with +gamma*bv
     bias -> out bf16, DMA out.
"""

import json
import sys

import ml_dtypes
import numpy as np

sys.path.insert(0, "/root/.axon_site")

from contextlib import ExitStack

import concourse.bass as bass
import concourse.bass2jax as b2j
import concourse.mybir as mybir
import concourse.tile as tile
from concourse.bass_utils import run_bass_kernel_spmd

F32 = mybir.dt.float32
BF16 = mybir.dt.bfloat16
AF = mybir.ActivationFunctionType
NE = mybir.AluOpType.not_equal
MUL = mybir.AluOpType.mult
ADD = mybir.AluOpType.add

B, C, H, W = 16, 512, 64, 64
S = H * W            # 4096
NB = 2               # batches per core
NCORES = 8

_PATCHED = False


def _install_multiwait_split():
    """This container's walrus rejects instructions carrying >1 sem waits.
    Split extras into standalone EventSemaphore waits on the same engine,
    inserted immediately before (preserves per-engine program order)."""
    global _PATCHED
    if _PATCHED:
        return
    _PATCHED = True
    orig = b2j._decompress_ant_bir

    def _split(s):
        d = json.loads(orig(s))
        for fn in d.get("functions", []):
            for blk in fn.get("blocks", []):
                out = []
                for ins in blk.get("instructions", []):
                    si = ins.get("sync_info")
                    ow = (si or {}).get("on_wait") or []
                    if len(ow) > 1:
                        for i, w in enumerate(ow[:-1]):
                            out.append({
                                "debug": ins.get("debug", 0),
                                "engine": ins["engine"],
                                "ins": [], "outs": [],
                                "name": f'{ins["name"]}-xw{i}',
                                "opcode": "EventSemaphore",
                                "sync_info": {"on_update": [], "on_wait": [w]},
                            })
                        si["on_wait"] = [ow[-1]]
                    out.append(ins)
                blk["instructions"] = out
        return json.dumps(d).encode()

    b2j._decompress_ant_bir = _split


def build_nc():
    nc = bass.Bass("TRN2", target_bir_lowering=False, debug=False)

    x_d = nc.dram_tensor("x", [NB, C, S], BF16, kind="ExternalInput").ap()
    wqk_d = nc.dram_tensor("wqkT", [C, 128], BF16, kind="ExternalInput").ap()
    bqk_d = nc.dram_tensor("bqk", [128, 1], F32, kind="ExternalInput").ap()
    wv_d = nc.dram_tensor("wvT", [C, C], BF16, kind="ExternalInput").ap()
    gbv_d = nc.dram_tensor("gbv", [C, 1], F32, kind="ExternalInput").ap()
    idf_d = nc.dram_tensor("identf", [64, 64], F32, kind="ExternalInput").ap()
    idb_d = nc.dram_tensor("identb", [128, 128], BF16, kind="ExternalInput").ap()
    out_d = nc.dram_tensor("out", [NB, C, S], BF16, kind="ExternalOutput").ap()
    vscr = [nc.dram_tensor(f"vscr{b}", [S, C], BF16, kind="Internal").ap()
            for b in range(NB)]
    oscr = [nc.dram_tensor(f"oscr{b}", [S, C], BF16, kind="Internal").ap()
            for b in range(NB)]

    x_v = x_d.rearrange("b (kc p) s -> b p kc s", p=128)
    out_v = out_d.rearrange("b (kc p) s -> b p kc s", p=128)
    wqk_v = wqk_d.rearrange("(kc p) m -> p kc m", p=128)
    wv_v = wv_d.rearrange("(kc p) m -> p kc m", p=128)
    gbv_v = gbv_d.rearrange("(kc p) one -> p kc one", p=128)

    with tile.TileContext(nc) as tc, ExitStack() as ctx:
        consts = ctx.enter_context(tc.tile_pool(name="consts", bufs=1))
        wqk_sb = consts.tile([128, 4, 128], BF16)
        nc.sync.dma_start(out=wqk_sb, in_=wqk_v)
        wv_sb = consts.tile([128, 4, 512], BF16)
        nc.sync.dma_start(out=wv_sb, in_=wv_v)
        bqk_sb = consts.tile([128, 1], F32)
        nc.sync.dma_start(out=bqk_sb, in_=bqk_d)
        gbv_sb = consts.tile([128, 4, 1], F32)
        nc.sync.dma_start(out=gbv_sb, in_=gbv_v)
        idf_sb = consts.tile([64, 64], F32)
        nc.sync.dma_start(out=idf_sb, in_=idf_d)
        idb_sb = consts.tile([128, 128], BF16)
        nc.sync.dma_start(out=idb_sb, in_=idb_d)

        # psum pools
        pps = ctx.enter_context(tc.tile_pool(name="pps", bufs=6, space="PSUM"))
        ppm = ctx.enter_context(tc.tile_pool(name="ppm", bufs=1, space="PSUM"))

        px = ctx.enter_context(tc.tile_pool(name="px", bufs=1))
        pqk = ctx.enter_context(tc.tile_pool(name="pqk", bufs=1))
        pP = ctx.enter_context(tc.tile_pool(name="pP", bufs=1))
        pscr = ctx.enter_context(tc.tile_pool(name="pscr", bufs=2))
        pst = ctx.enter_context(tc.tile_pool(name="pst", bufs=1))
        pvt = ctx.enter_context(tc.tile_pool(name="pvt", bufs=1))
        pcm = ctx.enter_context(tc.tile_pool(name="pcm", bufs=1))
        poc = ctx.enter_context(tc.tile_pool(name="poc", bufs=1))
        pop = ctx.enter_context(tc.tile_pool(name="pop", bufs=1))
        pstage = ctx.enter_context(tc.tile_pool(name="pstage", bufs=1))

        for b in range(NB):
            # ---- load x ----------------------------------------------------
            x_sb = px.tile([128, 4, S], BF16, tag="x")
            for kc in range(4):
                nc.sync.dma_start(out=x_sb[:, kc, :], in_=x_v[b, :, kc, :])

            # ---- qk projection --------------------------------------------
            qkA = pqk.tile([128, S], BF16, tag="qkA")
            qkB = pqk.tile([128, S], BF16, tag="qkB")
            for n in range(8):
                ps = pps.tile([128, 512], F32, tag="ps")
                for kc in range(4):
                    nc.tensor.matmul(
                        ps,
                        lhsT=wqk_sb[:, kc, :],
                        rhs=x_sb[:, kc, n * 512:(n + 1) * 512],
                        start=(kc == 0), stop=(kc == 3),
                    )
                nc.scalar.activation(
                    out=qkA[:, n * 512:(n + 1) * 512], in_=ps,
                    func=AF.Identity, bias=bqk_sb, scale=1.0,
                )
            # B = partition-swapped copy of A (k on top, q on bottom)
            nc.sync.dma_start(out=qkB[0:64, :], in_=qkA[64:128, :])
            nc.sync.dma_start(out=qkB[64:128, :], in_=qkA[0:64, :])

            # ---- vT row-major projection ----------------------------------
            vt_rm = pvt.tile([128, 32, 512], BF16, tag="vt")
            for j in range(32):
                ps = pps.tile([128, 512], F32, tag="ps")
                for kc in range(4):
                    nc.tensor.matmul(
                        ps, lhsT=x_sb[:, kc, j * 128:(j + 1) * 128],
                        rhs=wv_sb[:, kc, :],
                        start=(kc == 0), stop=(kc == 3))
                if j % 2 == 0:
                    nc.scalar.activation(out=vt_rm[:, j, :], in_=ps,
                                         func=AF.Identity)
                else:
                    nc.vector.tensor_copy(vt_rm[:, j, :], ps)

            # ---- vT col-major via DRAM round-trip -------------------------
            vs = vscr[b].rearrange("(t p) c -> p t c", p=128)
            nc.sync.dma_start(out=vs, in_=vt_rm)
            vt_cm = pcm.tile([128, 32, 512], BF16, tag="perm")
            # vt_cm[wr*64+h, wq, c] = vscr[h*64 + 2*wq + wr, c]
            vsp = vscr[b].rearrange("(h wq wr) c -> wr h wq c", wr=2, wq=32)
            for wr in range(2):
                nc.sync.dma_start(out=vt_cm[wr * 64:(wr + 1) * 64], in_=vsp[wr])

            Acol = qkA.rearrange("p (h w) -> p w h", w=W)
            Bcol = qkB.rearrange("p (h w) -> p w h", w=W)
            Arow = qkA.rearrange("p (h w) -> p h w", h=H)
            Brow = qkB.rearrange("p (h w) -> p h w", h=H)

            # ---- energies + softmax sums ----------------------------------
            PcolT = pP.tile([128, 2048], BF16, tag="PcolT")
            ProwT = pP.tile([128, 2048], BF16, tag="ProwT")
            Sc_p = pst.tile([128, 32], F32, tag="Scp")
            Sr_p = pst.tile([128, 32], F32, tag="Srp")

            for part in ("col", "row"):
                P_sb = PcolT if part == "col" else ProwT
                S_sb = Sc_p if part == "col" else Sr_p
                Ksrc = Bcol if part == "col" else Brow   # k in top half
                Qsrc = Acol if part == "col" else Arow   # q in top half
                for bi in range(4):
                    o2 = pps.tile([128, 512], F32, tag="ps")
                    o1 = pps.tile([128, 512], F32, tag="ps")
                    for sl in range(8):
                        m = bi * 8 + sl
                        w0, w1 = 2 * m, 2 * m + 1
                        fs = slice(sl * 64, (sl + 1) * 64)
                        # orientation-2: out [g, h] (resp. [u, w])
                        nc.tensor.matmul(o2[0:64, fs], lhsT=Ksrc[0:64, w0, :],
                                         rhs=Qsrc[0:64, w0, :], start=True, stop=True)
                        nc.tensor.matmul(o2[64:128, fs], lhsT=Qsrc[64:128, w1, :],
                                         rhs=Ksrc[64:128, w1, :], start=True, stop=True)
                        # orientation-1: out [h, g] (resp. [w, u])
                        nc.tensor.matmul(o1[0:64, fs], lhsT=Qsrc[0:64, w0, :],
                                         rhs=Ksrc[0:64, w0, :], start=True, stop=True)
                        nc.tensor.matmul(o1[64:128, fs], lhsT=Ksrc[64:128, w1, :],
                                         rhs=Qsrc[64:128, w1, :], start=True, stop=True)
                    bs = slice(bi * 512, (bi + 1) * 512)
                    nc.scalar.activation(out=P_sb[:, bs], in_=o2, func=AF.Exp)
                    scr = pscr.tile([128, 512], F32, tag="scr")
                    nc.scalar.activation(out=scr, in_=o1, func=AF.Exp)
                    if part == "col":
                        scr3 = scr.rearrange("p (m g) -> p m g", g=64)
                        nc.gpsimd.affine_select(
                            out=scr3[0:64], in_=scr3[0:64],
                            pattern=[[0, 8], [-1, 64]], compare_op=NE,
                            fill=0.0, base=0, channel_multiplier=1)
                        nc.gpsimd.affine_select(
                            out=scr3[64:128], in_=scr3[64:128],
                            pattern=[[0, 8], [-1, 64]], compare_op=NE,
                            fill=0.0, base=0, channel_multiplier=1)
                    nc.vector.reduce_sum(
                        out=S_sb[:, bi * 8:(bi + 1) * 8],
                        in_=scr.rearrange("p (m g) -> p m g", g=64),
                        axis=mybir.AxisListType.X)
                if part == "col":
                    P3 = P_sb.rearrange("p (m h) -> p m h", h=64)
                    nc.gpsimd.affine_select(
                        out=P3[0:64], in_=P3[0:64],
                        pattern=[[0, 32], [-1, 64]], compare_op=NE,
                        fill=0.0, base=0, channel_multiplier=1)
                    nc.gpsimd.affine_select(
                        out=P3[64:128], in_=P3[64:128],
                        pattern=[[0, 32], [-1, 64]], compare_op=NE,
                        fill=0.0, base=0, channel_multiplier=1)

            # ---- stats: R = 1/(Sc + Sr) in both pack layouts --------------
            Sc_pl = pst.tile([64, 64], F32, tag="Scpl")   # [h, w]
            Sr_pl = pst.tile([64, 64], F32, tag="Srpl")   # [w, h]
            Sc2 = Sc_pl.rearrange("p (m t) -> p m t", t=2)
            Sr2 = Sr_pl.rearrange("p (m t) -> p m t", t=2)
            nc.sync.dma_start(out=Sc2[:, :, 0], in_=Sc_p[0:64, :])
            nc.sync.dma_start(out=Sc2[:, :, 1], in_=Sc_p[64:128, :])
            nc.sync.dma_start(out=Sr2[:, :, 0], in_=Sr_p[0:64, :])
            nc.sync.dma_start(out=Sr2[:, :, 1], in_=Sr_p[64:128, :])
            tp = ppm.tile([64, 64], F32, tag="stps")
            nc.tensor.transpose(tp, Sr_pl, idf_sb)              # -> [h, w]
            R_hw = pst.tile([64, 64], F32, tag="Rhw")
            nc.vector.tensor_add(R_hw, Sc_pl, tp)
            nc.vector.reciprocal(out=R_hw, in_=R_hw)
            tp2 = ppm.tile([64, 64], F32, tag="stps")
            nc.tensor.transpose(tp2, R_hw, idf_sb)              # -> [w, h]
            R_wh = pst.tile([64, 64], F32, tag="Rwh")
            nc.vector.tensor_copy(R_wh, tp2)
            R_cs = pst.tile([128, 32], F32, tag="Rcs")
            R_rs = pst.tile([128, 32], F32, tag="Rrs")
            Rhw2 = R_hw.rearrange("p (m t) -> p m t", t=2)
            Rwh2 = R_wh.rearrange("p (m t) -> p m t", t=2)
            nc.sync.dma_start(out=R_cs[0:64, :], in_=Rhw2[:, :, 0])
            nc.sync.dma_start(out=R_cs[64:128, :], in_=Rhw2[:, :, 1])
            nc.sync.dma_start(out=R_rs[0:64, :], in_=Rwh2[:, :, 0])
            nc.sync.dma_start(out=R_rs[64:128, :], in_=Rwh2[:, :, 1])

            # ---- col aggregation (unscaled) -> ot_col ---------------------
            ot_col = poc.tile([128, 32, 512], BF16, tag="otcol")
            for m in range(32):
                aps = pps.tile([128, 512], F32, tag="ps")
                nc.tensor.matmul(aps[0:64, :],
                                 lhsT=PcolT[0:64, m * 64:(m + 1) * 64],
                                 rhs=vt_cm[0:64, m, :], start=True, stop=True)
                nc.tensor.matmul(aps[64:128, :],
                                 lhsT=PcolT[64:128, m * 64:(m + 1) * 64],
                                 rhs=vt_cm[64:128, m, :], start=True, stop=True)
                if m % 2 == 0:
                    nc.scalar.activation(out=ot_col[:, m, :], in_=aps,
                                         func=AF.Identity)
                else:
                    nc.vector.tensor_copy(ot_col[:, m, :], aps)

            # ---- permute col part to row-major order ----------------------
            # oscr[(h*64 + 2*wq + wr), c] = ot_col[wr*64+h, wq, c]
            osp = oscr[b].rearrange("(h wq wr) c -> wr h wq c", wr=2, wq=32)
            for wr in range(2):
                nc.sync.dma_start(out=osp[wr], in_=ot_col[wr * 64:(wr + 1) * 64])
            ot_colp = pop.tile([128, 32, 512], BF16, tag="otp")
            osv = oscr[b].rearrange("(t p) c -> p t c", p=128)
            for q in range(4):
                nc.sync.dma_start(out=ot_colp[:, q * 8:(q + 1) * 8, :],
                                  in_=osv[:, q * 8:(q + 1) * 8, :])

            # ---- row aggregation + col part folded in, scaled -> acc ------
            acc = ot_colp  # updated in place, slice by slice
            for m in range(32):
                aps = pps.tile([128, 512], F32, tag="ps")
                nc.tensor.matmul(aps[0:64, :],
                                 lhsT=ProwT[0:64, m * 64:(m + 1) * 64],
                                 rhs=vt_rm[0:64, m, :], start=True, stop=False,
                                 skip_group_check=True)
                nc.tensor.matmul(aps[64:128, :],
                                 lhsT=ProwT[64:128, m * 64:(m + 1) * 64],
                                 rhs=vt_rm[64:128, m, :], start=True, stop=False,
                                 skip_group_check=True)
                nc.tensor.matmul(aps, lhsT=idb_sb, rhs=ot_colp[:, m, :],
                                 start=False, stop=True, skip_group_check=True)
                if m % 2 == 0:
                    nc.scalar.activation(out=acc[:, m, :], in_=aps,
                                         func=AF.Copy, scale=R_rs[:, m:m + 1])
                else:
                    nc.vector.tensor_scalar_mul(out=acc[:, m, :], in0=aps,
                                                scalar1=R_rs[:, m:m + 1])

            # ---- final: out[c, s] = acc^T + x + gbv -----------------------
            for cc in range(4):
                stage = pstage.tile([128, S], BF16, tag="stage")
                for tb in range(8):
                    ps = pps.tile([128, 512], F32, tag="ps")
                    nc.tensor.matmul(
                        ps, lhsT=idb_sb,
                        rhs=x_sb[:, cc, tb * 512:(tb + 1) * 512],
                        start=True, stop=False, skip_group_check=True)
                    for mm in range(4):
                        t = tb * 4 + mm
                        nc.tensor.matmul(
                            ps[:, mm * 128:(mm + 1) * 128],
                            lhsT=acc[:, t, cc * 128:(cc + 1) * 128],
                            rhs=idb_sb, start=False, stop=(mm == 3),
                            skip_group_check=True)
                    fs = slice(tb * 512, (tb + 1) * 512)
                    if tb % 2 == 0:
                        nc.scalar.activation(out=stage[:, fs], in_=ps,
                                             func=AF.Identity,
                                             bias=gbv_sb[:, cc, :], scale=1.0)
                    else:
                        nc.vector.tensor_scalar_add(out=stage[:, fs], in0=ps,
                                                    scalar1=gbv_sb[:, cc, :])
                nc.sync.dma_start(out=out_v[b, :, cc, :], in_=stage)

    return nc


_NC = None


def _prep_consts(Wq, bq, Wk, bk, Wv, bv, gamma):
    wqkT = np.ascontiguousarray(
        np.concatenate([Wq, Wk], 0).T).astype(ml_dtypes.bfloat16)
    bqk = np.ascontiguousarray(
        np.concatenate([bq, bk])[:, None]).astype(np.float32)
    wvT = np.ascontiguousarray(
        (gamma[0] * Wv).T).astype(ml_dtypes.bfloat16)
    gbv = np.ascontiguousarray((gamma[0] * bv)[:, None]).astype(np.float32)
    identf = np.eye(64, dtype=np.float32)
    identb = np.eye(128, dtype=np.float32).astype(ml_dtypes.bfloat16)
    return wqkT, bqk, wvT, gbv, identf, identb


def kernel(x, Wq, bq, Wk, bk, Wv, bv, gamma):
    global _NC
    _install_multiwait_split()
    x = np.asarray(x, dtype=np.float32)
    Wq = np.asarray(Wq, np.float32); Wk = np.asarray(Wk, np.float32)
    Wv = np.asarray(Wv, np.float32)
    bq = np.asarray(bq, np.float32); bk = np.asarray(bk, np.float32)
    bv = np.asarray(bv, np.float32); gamma = np.asarray(gamma, np.float32)

    wqkT, bqk, wvT, gbv, identf, identb = _prep_consts(
        Wq, bq, Wk, bk, Wv, bv, gamma)

    if _NC is None:
        _NC = build_nc()

    xs = np.ascontiguousarray(x.reshape(B, C, S)).astype(ml_dtypes.bfloat16)
    in_maps = []
    for i in range(NCORES):
        in_maps.append({
            "x": np.ascontiguousarray(xs[i * NB:(i + 1) * NB]),
            "wqkT": wqkT, "bqk": bqk, "wvT": wvT, "gbv": gbv,
            "identf": identf, "identb": identb,
        })
    res = run_bass_kernel_spmd(_NC, in_maps, list(range(NCORES)))
    out = np.empty((B, C, S), np.float32)
    for i in range(NCORES):
        out[i * NB:(i + 1) * NB] = res.results[i]["out"].astype(np.float32)
    return out.reshape(B, C, H, W)
